# revision 2
# baseline (speedup 1.0000x reference)
"""MoE BatchedExperts kernel for 8 trn2 NeuronCores.

Strategy: expert parallelism with host-side top-k dispatch. Each token has
exactly TOP_K nonzero routing weights, so core e only processes the tokens
routed to expert e (~N*K/E of them) instead of all N — 4x less compute than
the dense reference formulation, identical math (zero-score tokens
contribute zero).

Per core e (tokens gathered+transposed on host to xT [D, T]):
  hT = gelu(w0[e].T-contracted mm: [F, T])      mm1: lhsT=w0 chunk, rhs=xT
  y  = (hT.T @ w1[e]) * scale                   mm2: lhsT=hT chunk, rhs=w1
Host combines: out[idx_e] += y_e rows; bias b1 folded in via routing @ b1.

All matmuls run as float32r (tf32-like, ~1e-4 rel err, full PE speed).
"""

import numpy as np

import concourse.bacc as bacc
import concourse.mybir as mybir
from concourse.tile import TileContext
from concourse.bass_utils import run_bass_kernel_spmd

F32 = mybir.dt.float32
F32R = mybir.dt.float32r

N, D, E, F = 4096, 1024, 8, 2048
P = 128
KD = D // P            # 8  k-tiles for mm1
KF = F // P            # 16 k-tiles for mm2
TCH = 384              # mm1 moving-dim chunk (>=256 keeps fp32r at full rate)
DCH = 256              # mm2 moving-dim chunk

_cache: dict[int, object] = {}


def build_program(T: int):
    """Bass program for one expert shard with T padded tokens."""
    assert T % TCH == 0 and T % P == 0
    NTC = T // TCH
    TO = T // P
    NDC = D // DCH

    nc = bacc.Bacc("TRN2", target_bir_lowering=False, debug=False)
    xT = nc.dram_tensor("xT", [D, T], F32R, kind="ExternalInput")
    w0 = nc.dram_tensor("w0", [D, F], F32R, kind="ExternalInput")
    w1 = nc.dram_tensor("w1", [F, D], F32R, kind="ExternalInput")
    b0 = nc.dram_tensor("b0", [F], F32, kind="ExternalInput")
    scale = nc.dram_tensor("scale", [T], F32, kind="ExternalInput")
    y = nc.dram_tensor("y", [T, D], F32, kind="ExternalOutput")

    xT_r = xT.rearrange("(ko p) t -> p ko t", p=P)
    w0_r = w0.rearrange("(ko p) f -> p ko f", p=P)
    w1_r = w1.rearrange("(ko p) d -> p ko d", p=P)

    with TileContext(nc) as tc:
        with tc.tile_pool(name="const", bufs=1) as const, \
             tc.tile_pool(name="xpool", bufs=1) as xpool, \
             tc.tile_pool(name="hpool", bufs=1) as hpool, \
             tc.tile_pool(name="w0pool", bufs=3) as w0pool, \
             tc.tile_pool(name="w1pool", bufs=2) as w1pool, \
             tc.tile_pool(name="ypool", bufs=4) as ypool, \
             tc.tile_pool(name="psum", bufs=8, space="PSUM") as psum:

            b0_sb = const.tile([P, KF], F32)
            nc.scalar.dma_start(b0_sb[:], b0.rearrange("(fo p) -> p fo", p=P))
            scale_sb = const.tile([P, TO], F32)
            nc.scalar.dma_start(scale_sb[:], scale.rearrange("(to p) -> p to", p=P))

            # x resident in SBUF, split by token chunk so mm1 can start early
            x_sb = []
            for t in range(NTC):
                xt = xpool.tile([P, KD, TCH], F32R, tag=f"x{t}")
                nc.scalar.dma_start(xt[:], xT_r[:, :, t * TCH:(t + 1) * TCH])
                x_sb.append(xt)

            # hT = gelu(x @ w0 + b0), laid out [F-part, T-free], fp32r
            h_sb = hpool.tile([P, KF, T], F32R)

            # ---- phase 1: mm1 + gelu ----
            for fo in range(KF):
                w0_sb = w0pool.tile([P, KD, P], F32R, tag="w0")
                nc.sync.dma_start(w0_sb[:], w0_r[:, :, fo * P:(fo + 1) * P])
                ps = [psum.tile([P, 512], F32, tag="ps", name=f"ps1_{fo}_{t}")[:, :TCH]
                      for t in range(NTC)]
                for k in range(KD):
                    for t in range(NTC):
                        nc.tensor.matmul(ps[t], w0_sb[:, k], x_sb[t][:, k],
                                         start=(k == 0), stop=(k == KD - 1))
                for t in range(NTC):
                    nc.scalar.activation(h_sb[:, fo, t * TCH:(t + 1) * TCH], ps[t],
                                         mybir.ActivationFunctionType.Gelu,
                                         bias=b0_sb[:, fo:fo + 1])

            # ---- phase 2: mm2 + scale ----
            for dc in range(NDC):
                w1_sb = w1pool.tile([P, KF, DCH], F32R, tag="w1")
                nc.scalar.dma_start(w1_sb[:], w1_r[:, :, dc * DCH:(dc + 1) * DCH])
                for to in range(TO):
                    ps2 = psum.tile([P, 512], F32, tag="ps", name=f"ps2_{dc}_{to}")[:, :DCH]
                    for k in range(KF):
                        nc.tensor.matmul(ps2, h_sb[:, k, to * P:(to + 1) * P],
                                         w1_sb[:, k],
                                         start=(k == 0), stop=(k == KF - 1))
                    y_sb = ypool.tile([P, DCH], F32, tag="y")
                    nc.vector.tensor_scalar_mul(y_sb[:], ps2, scale_sb[:, to:to + 1])
                    nc.sync.dma_start(
                        y[to * P:(to + 1) * P, dc * DCH:(dc + 1) * DCH], y_sb[:])

    nc.compile()
    return nc


def kernel(x, routing_tensor, w0, b0, w1, b1):
    x = np.ascontiguousarray(np.asarray(x, dtype=np.float32))
    routing = np.asarray(routing_tensor, dtype=np.float32)
    w0 = np.ascontiguousarray(np.asarray(w0, dtype=np.float32))
    b0 = np.asarray(b0, dtype=np.float32)
    w1 = np.ascontiguousarray(np.asarray(w1, dtype=np.float32))
    b1 = np.asarray(b1, dtype=np.float32)

    idx = [np.nonzero(routing[:, e])[0] for e in range(E)]
    cnt = [len(i) for i in idx]
    T = max(TCH, -(-max(cnt) // TCH) * TCH)

    nc = _cache.get(T)
    if nc is None:
        nc = _cache[T] = build_program(T)

    in_maps = []
    for e in range(E):
        xTe = np.zeros((D, T), dtype=np.float32)
        xTe[:, :cnt[e]] = x[idx[e]].T
        sc = np.zeros(T, dtype=np.float32)
        sc[:cnt[e]] = routing[idx[e], e]
        in_maps.append({"xT": xTe, "w0": w0[e], "w1": w1[e],
                        "b0": np.ascontiguousarray(b0[e, 0]), "scale": sc})

    res = run_bass_kernel_spmd(nc, in_maps, core_ids=list(range(E)))

    # combine: out = sum_e r_e * (y_e + b1_e); device already applied r_e * y_e
    out = routing @ b1[:, 0, :]
    for e in range(E):
        out[idx[e]] += res.results[e]["y"][:cnt[e]]
    return out.astype(np.float32)


# revision 3
# speedup vs baseline: 1.0169x; 1.0169x over previous
"""MoE BatchedExperts kernel for 8 trn2 NeuronCores.

Strategy: expert parallelism with host-side top-k dispatch. Each token has
exactly TOP_K nonzero routing weights, so core e only processes the tokens
routed to expert e (~N*K/E of them) instead of all N — 4x less compute than
the dense reference formulation, identical math (zero-score tokens
contribute zero).

Per core e (tokens gathered+transposed on host to xT [D, T]):
  hT = gelu(mm1 + b0)   [F, T]   mm1: lhsT=w0 chunk [128,128], rhs=xT chunk
  y  = hT.T @ w1[e]     [T, D]   mm2: lhsT=hT chunk [128,128], rhs=w1 chunk
Host combines: out[idx_e] += r_e * y_e rows; b1 folded in via routing @ b1.

All matmuls run as float32r (tf32-like, ~1e-4 rel err, full PE rate:
1 cycle/row warm). PE is the bottleneck; everything else overlaps.
"""

import numpy as np

import concourse.bacc as bacc
import concourse.mybir as mybir
from concourse.tile import TileContext
from concourse.bass_utils import run_bass_kernel_spmd

F32 = mybir.dt.float32
F32R = mybir.dt.float32r

N, D, E, F = 4096, 1024, 8, 2048
P = 128
KD = D // P            # 8  k-tiles for mm1
KF = F // P            # 16 k-tiles for mm2
TCH = 384              # mm1 moving-dim chunk (>=256 keeps fp32r at full rate)
D_CHUNKS = [384, 384, 256]   # mm2 moving-dim chunks (sum = D)
assert sum(D_CHUNKS) == D

_cache: dict[int, object] = {}


def build_program(T: int):
    """Bass program for one expert shard with T padded tokens."""
    assert T % TCH == 0 and T % P == 0
    NTC = T // TCH
    TO = T // P

    nc = bacc.Bacc("TRN2", target_bir_lowering=False, debug=False)
    xT = nc.dram_tensor("xT", [D, T], F32R, kind="ExternalInput")
    w0 = nc.dram_tensor("w0", [D, F], F32R, kind="ExternalInput")
    w1 = nc.dram_tensor("w1", [F, D], F32R, kind="ExternalInput")
    b0 = nc.dram_tensor("b0", [F], F32, kind="ExternalInput")
    y = nc.dram_tensor("y", [T, D], F32, kind="ExternalOutput")

    xT_r = xT.rearrange("(ko p) t -> p ko t", p=P)
    w0_r = w0.rearrange("(ko p) f -> p ko f", p=P)
    w1_r = w1.rearrange("(ko p) d -> p ko d", p=P)

    with TileContext(nc) as tc:
        with tc.tile_pool(name="const", bufs=1) as const, \
             tc.tile_pool(name="xpool", bufs=1) as xpool, \
             tc.tile_pool(name="hpool", bufs=1) as hpool, \
             tc.tile_pool(name="w0pool", bufs=4) as w0pool, \
             tc.tile_pool(name="w1pool", bufs=2) as w1pool, \
             tc.tile_pool(name="ypool", bufs=4) as ypool, \
             tc.tile_pool(name="psum", bufs=8, space="PSUM") as psum:

            # x resident in SBUF, tiled per (token-chunk, ko) so the first
            # matmul only waits for one 192KB transfer
            x_sb = []
            for t in range(NTC):
                row = []
                for k in range(KD):
                    xt = xpool.tile([P, TCH], F32R, tag=f"x{t}_{k}",
                                    name=f"x{t}_{k}")
                    nc.scalar.dma_start(xt[:], xT_r[:, k, t * TCH:(t + 1) * TCH])
                    row.append(xt)
                x_sb.append(row)

            b0_sb = const.tile([P, KF], F32)
            nc.scalar.dma_start(b0_sb[:], b0.rearrange("(fo p) -> p fo", p=P))

            # hT = gelu(x @ w0 + b0), laid out [F-part, T-free], fp32r
            h_sb = hpool.tile([P, KF, T], F32R)

            # ---- phase 1: mm1 + gelu ----
            for fo in range(KF):
                w0_sb = w0pool.tile([P, KD, P], F32R, tag="w0")
                nc.sync.dma_start(w0_sb[:], w0_r[:, :, fo * P:(fo + 1) * P])
                ps = [psum.tile([P, 512], F32, tag="ps", name=f"ps1_{fo}_{t}")[:, :TCH]
                      for t in range(NTC)]
                for k in range(KD):
                    for t in range(NTC):
                        nc.tensor.matmul(ps[t], w0_sb[:, k], x_sb[t][k],
                                         start=(k == 0), stop=(k == KD - 1))
                for t in range(NTC):
                    nc.scalar.activation(h_sb[:, fo, t * TCH:(t + 1) * TCH], ps[t],
                                         mybir.ActivationFunctionType.Gelu,
                                         bias=b0_sb[:, fo:fo + 1])

            # ---- phase 2: mm2 ----
            doff = 0
            for dc, DCH in enumerate(D_CHUNKS):
                w1_sb = w1pool.tile([P, KF, max(D_CHUNKS)], F32R, tag="w1",
                                    name=f"w1_{dc}")[:, :, :DCH]
                nc.scalar.dma_start(w1_sb[:], w1_r[:, :, doff:doff + DCH])
                for to in range(TO):
                    ps2 = psum.tile([P, 512], F32, tag="ps",
                                    name=f"ps2_{dc}_{to}")[:, :DCH]
                    for k in range(KF):
                        nc.tensor.matmul(ps2, h_sb[:, k, to * P:(to + 1) * P],
                                         w1_sb[:, k],
                                         start=(k == 0), stop=(k == KF - 1))
                    y_sb = ypool.tile([P, max(D_CHUNKS)], F32, tag="y",
                                      name=f"y_{dc}_{to}")[:, :DCH]
                    nc.vector.tensor_copy(y_sb, ps2)
                    nc.sync.dma_start(y[to * P:(to + 1) * P, doff:doff + DCH], y_sb)
                doff += DCH

    nc.compile()
    return nc


def kernel(x, routing_tensor, w0, b0, w1, b1):
    x = np.ascontiguousarray(np.asarray(x, dtype=np.float32))
    routing = np.asarray(routing_tensor, dtype=np.float32)
    w0 = np.ascontiguousarray(np.asarray(w0, dtype=np.float32))
    b0 = np.asarray(b0, dtype=np.float32)
    w1 = np.ascontiguousarray(np.asarray(w1, dtype=np.float32))
    b1 = np.asarray(b1, dtype=np.float32)

    idx = [np.nonzero(routing[:, e])[0] for e in range(E)]
    cnt = [len(i) for i in idx]
    T = max(TCH, -(-max(cnt) // TCH) * TCH)

    nc = _cache.get(T)
    if nc is None:
        nc = _cache[T] = build_program(T)

    in_maps = []
    for e in range(E):
        xTe = np.zeros((D, T), dtype=np.float32)
        xTe[:, :cnt[e]] = x[idx[e]].T
        in_maps.append({"xT": xTe, "w0": w0[e], "w1": w1[e],
                        "b0": np.ascontiguousarray(b0[e, 0])})

    res = run_bass_kernel_spmd(nc, in_maps, core_ids=list(range(E)))

    # combine: out = sum_e r_e * (y_e + b1_e)
    out = routing @ b1[:, 0, :]
    for e in range(E):
        r = routing[idx[e], e:e + 1]
        out[idx[e]] += r * res.results[e]["y"][:cnt[e]]
    return out.astype(np.float32)
